# revision 23
# baseline (speedup 1.0000x reference)
"""Trainium2 Bass kernel for nn_K_attention_ex (gaussian-kernel residual attention).

Reference computation (per batch sample b):
    sq_i   = ||x_i||^2
    G      = x @ x^T                      (T,T) gram
    sqdist = relu(sq_i + sq_j - 2 G)
    K      = exp(-sqdist * r + m) * (1 - eye)
    out    = x + K @ x

Algebraic restructuring (exact up to fp rounding):
    K_full = beta * e_i * e_j * exp(2 r g_ij),  e = exp(-r*sq), beta = exp(m)
    Diagonal of K_full is beta exactly, so zeroing it equals subtracting
    beta*x from K_full @ x:
    out = (1-beta)*x + beta * e ⊙_row ( E' @ x ),
      E'[p,i] = e_p * exp(2r G[p,i]) = exp(2r G[p,i] - r*sq_p)
    The e_p column factor is folded into the ACT exp as a per-partition bias,
    so the Y matmul consumes raw x (no scaled copy needed).

Sharding: data-parallel over batch B=16 across 8 NeuronCores (2 samples each,
processed sequentially per core; ACT-engine exp over the (T,T) kernel is the
bottleneck at ~1 elem/lane/cycle, so everything else is hidden under it).

Per-core dataflow (per sample):
    x_sb  (128,16,64)  natural layout, partition p = t%128, k = t//128
    xT    (64,2048)    via 16 PE transposes + DVE evacuation (K=64 gram, no
                       zero padding)
    sq -> ebias=-r*sq -> e -> f=beta*e, ax = alpha*x
    two passes q over YT column halves (psum budget: G pingpong 4 banks +
    YT half 2 banks + transpose scratch 2 banks = 8):
      for j in 0..15:
        G (128,1024) psum = 2 matmuls (N=512), lhsT = xT[:,jblk] (K=64)
        E (128,1024) sbuf = ACT exp(2r*G + ebias_j)
        YTq (64,1024) psum += x[jblk]-stationary @ E  (2 matmuls, N=512)
      YTq -> sbuf -> 8 PE transposes -> ytr psum -> DVE stt:
        out = ax + f ⊙_row Y ; DMA out per 4-block group.
    Prep for the next sample and the out-phase of the previous pass are
    emitted interleaved into the j-loop so ACT never idles at boundaries.
"""

import numpy as np

import concourse.bass as bass
import concourse.tile as tile
from concourse import bacc, mybir
from concourse.bass_utils import run_bass_kernel_spmd
from concourse.masks import make_identity

F32 = mybir.dt.float32
F32R = mybir.dt.float32r  # fp32 data, PE fast-fp32 matmul mode
AF = mybir.ActivationFunctionType
B, T, C = 16, 2048, 64
N_CORES = 8
BPC = B // N_CORES          # samples per core
NK = T // 128               # 16 row-blocks of 128


def build_nc(reps=1, stages='all'):
    nc = bacc.Bacc("TRN2", target_bir_lowering=False, debug=False, num_devices=N_CORES)
    x_in = nc.dram_tensor("x", [BPC, T, C], F32, kind="ExternalInput")
    r_in = nc.dram_tensor("r_sigma", [1], F32, kind="ExternalInput")
    m_in = nc.dram_tensor("margin", [1], F32, kind="ExternalInput")
    o_out = nc.dram_tensor("out", [BPC, T, C], F32, kind="ExternalOutput")

    with tile.TileContext(nc) as tc:
        if reps == 1:
            _body(tc, o_out.ap(), x_in.ap(), r_in.ap(), m_in.ap(), stages)
        else:
            with tc.For_i(0, reps, 1):
                _body(tc, o_out.ap(), x_in.ap(), r_in.ap(), m_in.ap(), stages)
    nc.compile()
    return nc


LEVELS = {'prep': 0, 'gram': 1, 'exp': 2, 'y': 3, 'all': 4}


def _body(tc, out_ap, x_ap, r_ap, m_ap, stages='all'):
    lvl = LEVELS[stages]
    nc = tc.nc
    with (
        tc.tile_pool(name="consts", bufs=1) as consts,
        tc.tile_pool(name="sx", bufs=2) as sx,
        tc.tile_pool(name="epool", bufs=3) as epool,
        tc.tile_pool(name="psG", bufs=2, space="PSUM") as psG,
        tc.tile_pool(name="psY", bufs=1, space="PSUM") as psY,
        tc.tile_pool(name="psT", bufs=1, space="PSUM") as psT,
        tc.tile_pool(name="psW", bufs=1, space="PSUM") as psW,
    ):
        # ---- one-time constants ----
        ident = consts.tile([128, 128], F32)
        make_identity(nc, ident)
        rb = consts.tile([128, 1], F32)
        nc.gpsimd.dma_start(out=rb, in_=r_ap.to_broadcast((128, 1)))
        mb = consts.tile([128, 1], F32)
        nc.gpsimd.dma_start(out=mb, in_=m_ap.to_broadcast((128, 1)))
        negr = consts.tile([128, 1], F32)
        nc.vector.tensor_scalar_mul(out=negr, in0=rb, scalar1=-1.0)
        s2r = consts.tile([128, 1], F32)
        nc.vector.tensor_scalar_mul(out=s2r, in0=rb, scalar1=2.0)
        beta = consts.tile([128, 1], F32)
        nc.scalar.activation(out=beta, in_=mb, func=AF.Exp)
        alpha = consts.tile([128, 1], F32)  # 1 - beta
        nc.vector.tensor_scalar(
            out=alpha, in0=beta, scalar1=-1.0, scalar2=1.0,
            op0=mybir.AluOpType.mult, op1=mybir.AluOpType.add,
        )
        ones_row = consts.tile([1, T], F32R)
        ones_stage = consts.tile([1, T], F32)
        nc.vector.memset(ones_stage, 1.0)
        nc.vector.tensor_copy(out=ones_row, in_=ones_stage)

        # prefetch both samples' inputs up front (per-ring DMA FIFOs run in
        # emission order; loads must not queue behind stores)
        x_sbs = []
        for s in range(BPC):
            xv = x_ap[s].rearrange("(p k) c -> p k c", p=128)
            x_sb = sx.tile([128, NK, C], F32, tag="x_sb", name=f"x_sb_{s}")
            nc.sync.dma_start(out=x_sb[:, 0:8, :], in_=xv[:, 0:8, :])
            nc.gpsimd.dma_start(out=x_sb[:, 8:NK, :], in_=xv[:, 8:NK, :])
            x_sbs.append(x_sb)

        # ---- PE warm-up / continuity scratch ----
        # The PE clock sits at 1.2 GHz until it sees ~3us of *continuous*
        # execution, and any stall resets the ramp. Two measures: (1) a
        # dependency-free fp32 burst at iteration start (long busy while the
        # input DMAs land), (2) dependency-free scratch matmuls woven through
        # the pipeline (emitted via pe_fill) so wait-prone instructions don't
        # reach the PE FIFO head before their dependencies resolve.
        scon = consts.tile([128, 256], F32R)
        scon_st = consts.tile([128, 256], F32)
        nc.vector.memset(scon_st, 0.5)
        nc.vector.tensor_copy(out=scon, in_=scon_st)
        scratch = psW.tile([64, 512], F32, tag="W", name="scratch")
        for i in range(12):
            nc.tensor.matmul(
                out=scratch[:, 0:128], lhsT=ident[:, 0:64], rhs=ident,
                start=True, stop=True,
            )

        def pe_fill(n=1):
            # dependency-free f32r matmul (N=256 stays on the fast path);
            # keeps the PE stream continuous across semaphore waits
            for _ in range(n):
                nc.tensor.matmul(
                    out=scratch[:, 0:256], lhsT=scon[:, 0:64], rhs=scon,
                    start=True, stop=True,
                )

        # ---- per-sample prep, emitted as filler closures ----
        # returns (tiles dict, list of closures to emit)
        def make_prep(s):
            x_sb = x_sbs[s]
            d = {}
            # Gram operand tiles; the exp argument comes out of the PE
            # finished (no ACT scale/bias APs, which cost ~270ns/instr):
            #   xTr (rhs):  rows 0-63 = x^T, row 64 = 1, rows 65-127 = 0
            #   xTl (lhsT): rows 0-63 = 2r*x^T, row 64 = -r*sq^T, rows 65-127
            #               garbage (multiplied by xTr's zero rows)
            #   G' = xTl^T @ xTr = 2r*G[p,i] - r*sq_p  -> E = exp(G')
            d['xTr'] = sx.tile([128, T], F32R, tag="xTr", name=f"xTr_{s}")
            d['xTl'] = sx.tile([128, T], F32R, tag="xTl", name=f"xTl_{s}")
            d['xr'] = sx.tile([128, NK, C], F32R, tag="xr", name=f"xr_{s}")
            d['sq'] = sx.tile([128, NK], F32, tag="sq", name=f"sq_{s}")
            d['ebias'] = sx.tile([128, NK], F32, tag="ebias", name=f"ebias_{s}")
            d['f'] = sx.tile([128, NK], F32, tag="f", name=f"f_{s}")
            d['ax'] = sx.tile([128, NK, C], F32, tag="ax", name=f"ax_{s}")
            ops = []

            def xt_zero():
                # zero xTr rows 64-127; source is a const broadcast so this
                # can run at any time (row 64 is then overwritten with ones)
                nc.vector.tensor_scalar_mul(
                    out=d['xTr'][64:128, :],
                    in0=ident[64:128, 0:1].to_broadcast((64, T)),
                    scalar1=0.0,
                )
                nc.gpsimd.dma_start(out=d['xTr'][64:65, :], in_=ones_row)
            ops.append(xt_zero)

            def xt_group(g):
                def emit():
                    pe_fill(2)
                    xtr = psT.tile([64, 4, 128], F32, tag="T", name=f"xtr_{s}_{g}")
                    for kk in range(4):
                        k = 4 * g + kk
                        nc.tensor.transpose(
                            out=xtr[:, kk, :], in_=x_sb[:, k, :], identity=ident
                        )
                    nc.vector.tensor_copy(
                        out=d['xTr'][:64, 512 * g: 512 * (g + 1)],
                        in_=xtr.rearrange("p a b -> p (a b)"),
                    )
                    nc.vector.tensor_scalar_mul(
                        out=d['xTl'][:64, 512 * g: 512 * (g + 1)],
                        in0=d['xTr'][:64, 512 * g: 512 * (g + 1)],
                        scalar1=s2r[0:64],
                    )
                return emit
            for g in range(4):
                ops.append(xt_group(g))

            def scalars():
                xsq = sx.tile([128, NK, C], F32, tag="xsq", name=f"xsq_{s}")
                nc.vector.tensor_mul(xsq, x_sb, x_sb)
                nc.vector.reduce_sum(out=d['sq'], in_=xsq, axis=mybir.AxisListType.X)
                nc.vector.tensor_scalar_mul(out=d['ebias'], in0=d['sq'], scalar1=negr)
                e = sx.tile([128, NK], F32, tag="e", name=f"e_{s}")
                nc.scalar.activation(out=e, in_=d['ebias'], func=AF.Exp)
                nc.vector.tensor_scalar_mul(out=d['f'], in0=e, scalar1=beta)
            ops.append(scalars)

            def ebrow():
                # xTl row 64 = -r*sq^T via PE transpose + DMA row collapse
                pe_fill(2)
                ebt = psT.tile([16, 128], F32, tag="T", name=f"ebt_{s}")
                nc.tensor.transpose(out=ebt, in_=d['ebias'], identity=ident)
                ebsb = sx.tile([16, 128], F32R, tag="ebsb", name=f"ebsb_{s}")
                nc.vector.tensor_copy(out=ebsb, in_=ebt)
                nc.gpsimd.dma_start(out=d['xTl'][64:65, :], in_=ebsb)
            ops.append(ebrow)

            def axop():
                nc.vector.tensor_scalar_mul(out=d['ax'], in0=x_sbs[s], scalar1=alpha)
            ops.append(axop)

            def xrop():
                nc.vector.tensor_copy(out=d['xr'], in_=x_sb)
            ops.append(xrop)
            return d, ops

        # ---- out-phase for one YT column half, as filler closures ----
        def make_out(s, q, YTq, prep):
            ov = out_ap[s].rearrange("(p k) c -> p k c", p=128)
            YTsb = sx.tile([64, 1024], F32, tag="YTsb", name=f"YTsb_{s}_{q}")
            ops = []

            def evac(h):
                def emit():
                    nc.vector.tensor_copy(
                        out=YTsb[:, 512 * h: 512 * (h + 1)],
                        in_=YTq[:, 512 * h: 512 * (h + 1)],
                    )
                return emit
            ops.append(evac(0))
            ops.append(evac(1))

            dma_legs = [nc.scalar, nc.sync]

            def outgroup(g):
                # g in 0,1 : blocks k = 8q + 4g + (0..3)
                def emit():
                    pe_fill(2)
                    ytr = psT.tile([128, 4, C], F32, tag="T", name=f"ytr_{s}_{q}_{g}")
                    for kk in range(4):
                        nc.tensor.transpose(
                            out=ytr[:, kk, :],
                            in_=YTsb[:, 128 * (4 * g + kk): 128 * (4 * g + kk + 1)],
                            identity=ident[:64, :64],
                        )
                    outsb = sx.tile([128, 4, C], F32, tag="outsb",
                                    name=f"outsb_{s}_{q}_{g}")
                    for kk in range(4):
                        k = 8 * q + 4 * g + kk
                        nc.vector.scalar_tensor_tensor(
                            out=outsb[:, kk, :], in0=ytr[:, kk, :],
                            scalar=prep['f'][:, k: k + 1],
                            in1=prep['ax'][:, k, :], op0=mybir.AluOpType.mult,
                            op1=mybir.AluOpType.add,
                        )
                    dma_legs[g].dma_start(
                        out=ov[:, 8 * q + 4 * g: 8 * q + 4 * g + 4, :], in_=outsb
                    )
                return emit
            ops.append(outgroup(0))
            ops.append(outgroup(1))
            return ops

        # ---- main: sequential samples, 2 YT column passes each ----
        filler = []   # deque of closures to interleave into j-loops

        def drain(n):
            for _ in range(min(n, len(filler))):
                filler.pop(0)()

        prep0, ops0 = make_prep(0)
        for op in ops0:
            op()          # first sample prep emitted immediately
        preps = {0: prep0}

        for s in range(BPC):
            prep = preps[s]
            xTl, xTr = prep['xTl'], prep['xTr']
            x_sb = x_sbs[s]
            if s + 1 < BPC:
                preps[s + 1], nops = make_prep(s + 1)
                filler.extend(nops)

            for q in range(2):
                YTq = psY.tile([64, 1024], F32, tag="YT", name=f"YT_{s}_{q}")
                cbase = 1024 * q

                def emit_gram(j):
                    lhsT_g = xTl[:, 128 * j: 128 * (j + 1)]
                    G = psG.tile([128, 1024], F32, tag="G", name=f"G_{s}_{q}_{j}")
                    for h in range(2):
                        n0 = cbase + 512 * h
                        nc.tensor.matmul(
                            out=G[:, 512 * h: 512 * (h + 1)],
                            lhsT=lhsT_g,
                            rhs=xTr[:, n0: n0 + 512],
                            start=True,
                            stop=True,
                        )
                    return G

                def emit_exp(G, j):
                    E = epool.tile([128, 1024], F32R, tag="E", name=f"E_{s}_{q}_{j}")
                    nc.scalar.activation(out=E, in_=G, func=AF.Exp)
                    return E

                def emit_y(E, j):
                    for h in range(2):
                        nc.tensor.matmul(
                            out=YTq[:, 512 * h: 512 * (h + 1)],
                            lhsT=prep['xr'][:, j, :],
                            rhs=E[:, 512 * h: 512 * (h + 1)],
                            start=(j == 0),
                            stop=(j == NK - 1),
                        )

                if lvl >= 1:
                    G = emit_gram(0)
                    for j in range(NK):
                        E = emit_exp(G, j) if lvl >= 2 else None
                        if j + 1 < NK:
                            pe_fill(1)
                            G = emit_gram(j + 1)
                        if lvl >= 3:
                            pe_fill(1)
                            emit_y(E, j)
                        drain(1)

                if lvl >= 4:
                    filler.extend(make_out(s, q, YTq, prep))

        # drain any remaining filler (last pass's out-phase)
        drain(len(filler) + 1)


_NC_CACHE = {}


def _get_nc(reps=1, stages='all'):
    key = (reps, stages)
    if key not in _NC_CACHE:
        _NC_CACHE[key] = build_nc(reps, stages)
    return _NC_CACHE[key]


def _run(x, r_sigma, margin, trace=False, reps=1, stages='all'):
    nc = _get_nc(reps, stages)
    x = np.ascontiguousarray(np.asarray(x, dtype=np.float32))
    r_sigma = np.ascontiguousarray(np.asarray(r_sigma, dtype=np.float32))
    margin = np.ascontiguousarray(np.asarray(margin, dtype=np.float32))
    in_maps = [
        {
            "x": np.ascontiguousarray(x[c * BPC: (c + 1) * BPC]),
            "r_sigma": r_sigma,
            "margin": margin,
        }
        for c in range(N_CORES)
    ]
    res = run_bass_kernel_spmd(nc, in_maps, core_ids=list(range(N_CORES)), trace=trace)
    out = np.concatenate([res.results[c]["out"] for c in range(N_CORES)], axis=0)
    return out, res


def kernel(x, r_sigma, margin):
    out, _ = _run(x, r_sigma, margin, trace=False)
    return out


# revision 25
# speedup vs baseline: 1.3476x; 1.3476x over previous
"""Trainium2 Bass kernel for nn_K_attention_ex (gaussian-kernel residual attention).

Reference computation (per batch sample b):
    sq_i   = ||x_i||^2
    G      = x @ x^T                      (T,T) gram
    sqdist = relu(sq_i + sq_j - 2 G)
    K      = exp(-sqdist * r + m) * (1 - eye)
    out    = x + K @ x

Algebraic restructuring (exact up to fp rounding):
    K_full = beta * e_i * e_j * exp(2 r g_ij),  e = exp(-r*sq), beta = exp(m)
    Diagonal of K_full is beta exactly, so zeroing it equals subtracting
    beta*x from K_full @ x:
    out = (1-beta)*x + beta * e ⊙_row ( E' @ x ),
      E'[p,i] = e_p * exp(2r G[p,i]) = exp(2r G[p,i] - r*sq_p)
    The e_p column factor is folded into the ACT exp as a per-partition bias,
    so the Y matmul consumes raw x (no scaled copy needed).

Sharding: data-parallel over batch B=16 across 8 NeuronCores (2 samples each,
processed sequentially per core; ACT-engine exp over the (T,T) kernel is the
bottleneck at ~1 elem/lane/cycle, so everything else is hidden under it).

Per-core dataflow (per sample):
    x_sb  (128,16,64)  natural layout, partition p = t%128, k = t//128
    xT    (64,2048)    via 16 PE transposes + DVE evacuation (K=64 gram, no
                       zero padding)
    sq -> ebias=-r*sq -> e -> f=beta*e, ax = alpha*x
    two passes q over YT column halves (psum budget: G pingpong 4 banks +
    YT half 2 banks + transpose scratch 2 banks = 8):
      for j in 0..15:
        G (128,1024) psum = 2 matmuls (N=512), lhsT = xT[:,jblk] (K=64)
        E (128,1024) sbuf = ACT exp(2r*G + ebias_j)
        YTq (64,1024) psum += x[jblk]-stationary @ E  (2 matmuls, N=512)
      YTq -> sbuf -> 8 PE transposes -> ytr psum -> DVE stt:
        out = ax + f ⊙_row Y ; DMA out per 4-block group.
    Prep for the next sample and the out-phase of the previous pass are
    emitted interleaved into the j-loop so ACT never idles at boundaries.
"""

import numpy as np

import concourse.bass as bass
import concourse.tile as tile
from concourse import bacc, mybir
from concourse.bass_utils import run_bass_kernel_spmd
from concourse.masks import make_identity

F32 = mybir.dt.float32
F32R = mybir.dt.float32r  # fp32 data, PE fast-fp32 matmul mode
AF = mybir.ActivationFunctionType
B, T, C = 16, 2048, 64
N_CORES = 8
BPC = B // N_CORES          # samples per core
NK = T // 128               # 16 row-blocks of 128


def build_nc(reps=1, stages='all'):
    nc = bacc.Bacc("TRN2", target_bir_lowering=False, debug=False, num_devices=N_CORES)
    x_in = nc.dram_tensor("x", [BPC, T, C], F32, kind="ExternalInput")
    r_in = nc.dram_tensor("r_sigma", [1], F32, kind="ExternalInput")
    m_in = nc.dram_tensor("margin", [1], F32, kind="ExternalInput")
    o_out = nc.dram_tensor("out", [BPC, T, C], F32, kind="ExternalOutput")

    with tile.TileContext(nc) as tc:
        if reps == 1:
            _body(tc, o_out.ap(), x_in.ap(), r_in.ap(), m_in.ap(), stages)
        else:
            with tc.For_i(0, reps, 1):
                _body(tc, o_out.ap(), x_in.ap(), r_in.ap(), m_in.ap(), stages)
    nc.compile()
    return nc


LEVELS = {'prep': 0, 'gram': 1, 'exp': 2, 'y': 3, 'all': 4}


def _body(tc, out_ap, x_ap, r_ap, m_ap, stages='all'):
    lvl = LEVELS[stages]
    nc = tc.nc
    with (
        tc.tile_pool(name="consts", bufs=1) as consts,
        tc.tile_pool(name="sx", bufs=2) as sx,
        tc.tile_pool(name="epool", bufs=3) as epool,
        tc.tile_pool(name="psG", bufs=2, space="PSUM") as psG,
        tc.tile_pool(name="psY", bufs=1, space="PSUM") as psY,
        tc.tile_pool(name="psT", bufs=2, space="PSUM") as psT,
    ):
        # ---- one-time constants ----
        ident = consts.tile([128, 128], F32)
        make_identity(nc, ident)
        rb = consts.tile([128, 1], F32)
        nc.gpsimd.dma_start(out=rb, in_=r_ap.to_broadcast((128, 1)))
        mb = consts.tile([128, 1], F32)
        nc.gpsimd.dma_start(out=mb, in_=m_ap.to_broadcast((128, 1)))
        negr = consts.tile([128, 1], F32)
        nc.vector.tensor_scalar_mul(out=negr, in0=rb, scalar1=-1.0)
        s2r = consts.tile([128, 1], F32)
        nc.vector.tensor_scalar_mul(out=s2r, in0=rb, scalar1=2.0)
        beta = consts.tile([128, 1], F32)
        nc.scalar.activation(out=beta, in_=mb, func=AF.Exp)
        alpha = consts.tile([128, 1], F32)  # 1 - beta
        nc.vector.tensor_scalar(
            out=alpha, in0=beta, scalar1=-1.0, scalar2=1.0,
            op0=mybir.AluOpType.mult, op1=mybir.AluOpType.add,
        )

        # prefetch both samples' inputs up front (per-ring DMA FIFOs run in
        # emission order; loads must not queue behind stores)
        x_sbs = []
        for s in range(BPC):
            xv = x_ap[s].rearrange("(p k) c -> p k c", p=128)
            x_sb = sx.tile([128, NK, C], F32, tag="x_sb", name=f"x_sb_{s}")
            nc.sync.dma_start(out=x_sb[:, 0:8, :], in_=xv[:, 0:8, :])
            nc.gpsimd.dma_start(out=x_sb[:, 8:NK, :], in_=xv[:, 8:NK, :])
            x_sbs.append(x_sb)

        # ---- per-sample prep, emitted as filler closures ----
        # returns (tiles dict, list of closures to emit)
        def make_prep(s):
            x_sb = x_sbs[s]
            d = {}
            # Gram operand tiles for row-tiled (K=64 x 2 concurrent) grams.
            # Both tiles carry x^T duplicated into partition halves 0-63 and
            # 64-127 so row-group 0 computes G_j while row-group 64 computes
            # G_{j+1} in the same pass. xTl is pre-scaled by 2r so the exp
            # needs no scale AP: G' = (2r x)^T x = 2r G.
            d['xTr'] = sx.tile([128, T], F32R, tag="xTr", name=f"xTr_{s}")
            d['xTl'] = sx.tile([128, T], F32R, tag="xTl", name=f"xTl_{s}")
            d['xs'] = sx.tile([128, NK, C], F32R, tag="xs", name=f"xs_{s}")
            d['sq'] = sx.tile([128, NK], F32, tag="sq", name=f"sq_{s}")
            d['ebias'] = sx.tile([128, NK], F32, tag="ebias", name=f"ebias_{s}")
            d['e'] = sx.tile([128, NK], F32, tag="e", name=f"e_{s}")
            d['f'] = sx.tile([128, NK], F32, tag="f", name=f"f_{s}")
            d['ax'] = sx.tile([128, NK, C], F32, tag="ax", name=f"ax_{s}")
            ops = []

            def xt_group(g):
                def emit():
                    xtr = psT.tile([64, 4, 128], F32, tag="T", name=f"xtr_{s}_{g}")
                    for kk in range(4):
                        k = 4 * g + kk
                        nc.tensor.transpose(
                            out=xtr[:, kk, :], in_=x_sb[:, k, :], identity=ident
                        )
                    cols = slice(512 * g, 512 * (g + 1))
                    nc.vector.tensor_copy(
                        out=d['xTr'][:64, cols],
                        in_=xtr.rearrange("p a b -> p (a b)"),
                    )
                    nc.vector.tensor_copy(
                        out=d['xTr'][64:128, cols], in_=d['xTr'][:64, cols],
                    )
                    nc.vector.tensor_scalar_mul(
                        out=d['xTl'][:, cols], in0=d['xTr'][:, cols], scalar1=s2r,
                    )
                return emit
            for g in range(4):
                ops.append(xt_group(g))

            def scalars():
                xsq = sx.tile([128, NK, C], F32, tag="xsq", name=f"xsq_{s}")
                nc.vector.tensor_mul(xsq, x_sb, x_sb)
                nc.vector.reduce_sum(out=d['sq'], in_=xsq, axis=mybir.AxisListType.X)
                nc.vector.tensor_scalar_mul(out=d['ebias'], in0=d['sq'], scalar1=negr)
                nc.scalar.activation(out=d['e'], in_=d['ebias'], func=AF.Exp)
                nc.vector.tensor_scalar_mul(out=d['f'], in0=d['e'], scalar1=beta)
            ops.append(scalars)

            def xsop(half):
                def emit():
                    for k in range(8 * half, 8 * half + 8):
                        nc.vector.tensor_scalar_mul(
                            out=d['xs'][:, k, :], in0=x_sb[:, k, :],
                            scalar1=d['e'][:, k: k + 1],
                        )
                return emit
            ops.append(xsop(0))
            ops.append(xsop(1))

            def axop():
                nc.vector.tensor_scalar_mul(out=d['ax'], in0=x_sbs[s], scalar1=alpha)
            ops.append(axop)
            return d, ops

        # ---- out-phase for one YT column half, as filler closures ----
        def make_out(s, q, YTq, prep):
            ov = out_ap[s].rearrange("(p k) c -> p k c", p=128)
            YTsb = sx.tile([64, 1024], F32, tag="YTsb", name=f"YTsb_{s}_{q}")
            ops = []

            def evac(h):
                def emit():
                    nc.vector.tensor_copy(
                        out=YTsb[:, 512 * h: 512 * (h + 1)],
                        in_=YTq[:, 512 * h: 512 * (h + 1)],
                    )
                return emit
            ops.append(evac(0))
            ops.append(evac(1))

            dma_legs = [nc.scalar, nc.sync]

            def outgroup(g):
                # g in 0,1 : blocks k = 8q + 4g + (0..3)
                def emit():
                    ytr = psT.tile([128, 4, C], F32, tag="T", name=f"ytr_{s}_{q}_{g}")
                    for kk in range(4):
                        nc.tensor.transpose(
                            out=ytr[:, kk, :],
                            in_=YTsb[:, 128 * (4 * g + kk): 128 * (4 * g + kk + 1)],
                            identity=ident[:64, :64],
                        )
                    outsb = sx.tile([128, 4, C], F32, tag="outsb",
                                    name=f"outsb_{s}_{q}_{g}")
                    for kk in range(4):
                        k = 8 * q + 4 * g + kk
                        nc.vector.scalar_tensor_tensor(
                            out=outsb[:, kk, :], in0=ytr[:, kk, :],
                            scalar=prep['f'][:, k: k + 1],
                            in1=prep['ax'][:, k, :], op0=mybir.AluOpType.mult,
                            op1=mybir.AluOpType.add,
                        )
                    dma_legs[g].dma_start(
                        out=ov[:, 8 * q + 4 * g: 8 * q + 4 * g + 4, :], in_=outsb
                    )
                return emit
            ops.append(outgroup(0))
            ops.append(outgroup(1))
            return ops

        # ---- main: sequential samples, 2 YT column passes each ----
        filler = []   # deque of closures to interleave into j-loops

        def drain(n):
            for _ in range(min(n, len(filler))):
                filler.pop(0)()

        prep0, ops0 = make_prep(0)
        for op in ops0:
            op()          # first sample prep emitted immediately
        preps = {0: prep0}

        for s in range(BPC):
            prep = preps[s]
            xTl, xTr = prep['xTl'], prep['xTr']
            x_sb = x_sbs[s]
            if s + 1 < BPC:
                preps[s + 1], nops = make_prep(s + 1)
                filler.extend(nops)

            for q in range(2):
                YTq = psY.tile([64, 1024], F32, tag="YT", name=f"YT_{s}_{q}")
                cbase = 1024 * q

                # step st = (jpair, c): row-group 0 computes G_{2jp}[:, c-chunk]
                # while row-group 64 concurrently computes G_{2jp+1}[:, c-chunk]
                def emit_gram(st):
                    jp, c = st >> 1, st & 1
                    j0, j1 = 2 * jp, 2 * jp + 1
                    n0 = cbase + 512 * c
                    G = psG.tile([128, 1024], F32, tag="G", name=f"G_{s}_{q}_{st}")
                    nc.tensor.matmul(
                        out=G[:, 0:512],
                        lhsT=xTl[0:64, 128 * j0: 128 * (j0 + 1)],
                        rhs=xTr[0:64, n0: n0 + 512],
                        start=True, stop=True, tile_position=(0, 0),
                    )
                    nc.tensor.matmul(
                        out=G[:, 512:1024],
                        lhsT=xTl[64:128, 128 * j1: 128 * (j1 + 1)],
                        rhs=xTr[64:128, n0: n0 + 512],
                        start=True, stop=True, tile_position=(64, 0),
                    )
                    return G

                def emit_exp(G, st):
                    E = epool.tile([128, 1024], F32R, tag="E", name=f"E_{s}_{q}_{st}")
                    nc.scalar.activation(out=E, in_=G, func=AF.Exp)
                    return E

                def emit_y(E, st):
                    jp, c = st >> 1, st & 1
                    j0, j1 = 2 * jp, 2 * jp + 1
                    yslice = YTq[:, 512 * c: 512 * (c + 1)]
                    nc.tensor.matmul(
                        out=yslice, lhsT=prep['xs'][:, j0, :], rhs=E[:, 0:512],
                        start=(jp == 0), stop=False,
                    )
                    nc.tensor.matmul(
                        out=yslice, lhsT=prep['xs'][:, j1, :], rhs=E[:, 512:1024],
                        start=False, stop=(jp == 7),
                    )

                if lvl >= 1:
                    G = emit_gram(0)
                    for st in range(NK):
                        E = emit_exp(G, st) if lvl >= 2 else None
                        if st + 1 < NK:
                            G = emit_gram(st + 1)
                        if lvl >= 3:
                            emit_y(E, st)
                        drain(1)

                if lvl >= 4:
                    filler.extend(make_out(s, q, YTq, prep))

        # drain any remaining filler (last pass's out-phase)
        drain(len(filler) + 1)


_NC_CACHE = {}


def _get_nc(reps=1, stages='all'):
    key = (reps, stages)
    if key not in _NC_CACHE:
        _NC_CACHE[key] = build_nc(reps, stages)
    return _NC_CACHE[key]


def _run(x, r_sigma, margin, trace=False, reps=1, stages='all'):
    nc = _get_nc(reps, stages)
    x = np.ascontiguousarray(np.asarray(x, dtype=np.float32))
    r_sigma = np.ascontiguousarray(np.asarray(r_sigma, dtype=np.float32))
    margin = np.ascontiguousarray(np.asarray(margin, dtype=np.float32))
    in_maps = [
        {
            "x": np.ascontiguousarray(x[c * BPC: (c + 1) * BPC]),
            "r_sigma": r_sigma,
            "margin": margin,
        }
        for c in range(N_CORES)
    ]
    res = run_bass_kernel_spmd(nc, in_maps, core_ids=list(range(N_CORES)), trace=trace)
    out = np.concatenate([res.results[c]["out"] for c in range(N_CORES)], axis=0)
    return out, res


def kernel(x, r_sigma, margin):
    out, _ = _run(x, r_sigma, margin, trace=False)
    return out
